# revision 13
# baseline (speedup 1.0000x reference)
"""Trainium2 Bass kernel for the BayesianOddLayer (neural BP decoder odd layer).

Structure exploited: w_odd2even_mask is block-diagonal with 4x4 blocks (edges
grouped by variable node, DV=4) and w_skipconn2even_mask maps variable v to its
4 incident edges.  The two [B,E]x[E,E] matmuls therefore collapse to per-window
dense matmuls: for each 64-column window w of the output, the only contributing
inputs are the 64 matching columns of x plus the 16 matching columns of llr.
We stack those into a K=80 stationary operand so a single fp32 TensorE matmul
per (batch-block, window, branch) produces msgs + llr_term directly in natural
[batch, edge] layout in PSUM.  ScalarE applies tanh(0.5*x) straight out of
PSUM; the +-10 pre-tanh clip is applied post-tanh as a fused min/max against
+-tanh(5) (tanh is monotone, so the two are equivalent).  The KL term's
row-sums of odd_weights**2 are computed on-device with Square+accum_out.

Sharding: data-parallel over batch across 8 cores (1024 rows/core); the tiny
masked-weight packs are replicated; odd_weights rows for the KL reduction are
row-sharded 256/core.
"""

import os

import numpy as np

N_VARS = 512
DV = 4
E = N_VARS * DV            # 2048
B = 8192
N_CORES = 8
B_CORE = B // N_CORES      # 1024
WIN = 64                   # output window (16 variable groups)
N_WIN = E // WIN           # 32
KROWS = WIN + WIN // DV    # 80 = 64 x-rows + 16 llr-rows
KL_ROWS = E // N_CORES     # 256 odd_weights rows per core
CLIP_TANH = 10.0
KL_SCALE = 5.0

_CACHED = None       # built Bass program
LAST_RESULT = None   # BassKernelResults of the most recent run (for profiling)
LAST_IN_MAPS = None  # per-core input dicts of the most recent run


def _build_bass(repeat=1):
    import concourse.bass as bass
    import concourse.bacc as bacc
    import concourse.tile as tile
    from concourse import mybir

    f32 = mybir.dt.float32
    nc = bacc.Bacc("TRN2", target_bir_lowering=False)

    xl = nc.dram_tensor("xl", [N_WIN * KROWS, B_CORE], f32, kind="ExternalInput")
    rz = nc.dram_tensor("rhs_z", [KROWS, E], f32, kind="ExternalInput")
    rzt = nc.dram_tensor("rhs_zt", [KROWS, E], f32, kind="ExternalInput")
    klw = nc.dram_tensor("klw", [KL_ROWS, E], f32, kind="ExternalInput")
    oz = nc.dram_tensor("oz", [B_CORE, E], f32, kind="ExternalOutput")
    ozt = nc.dram_tensor("ozt", [B_CORE, E], f32, kind="ExternalOutput")
    klp = nc.dram_tensor("klp", [128, 8], f32, kind="ExternalOutput")

    TANH5 = float(np.tanh(0.5 * CLIP_TANH))
    QCOLS = B_CORE // 4  # 256 batch columns per resident xl quarter

    with tile.TileContext(nc) as tc:
        with (
            tc.tile_pool(name="singles", bufs=1) as singles,
            tc.tile_pool(name="xq", bufs=2) as xqp,
            tc.tile_pool(name="klt", bufs=2) as klt,
            tc.tile_pool(name="outs", bufs=2) as outs,
            tc.tile_pool(name="psum", bufs=2, space="PSUM") as psum,
        ):
            rz_sb = singles.tile([KROWS, E], f32, tag="rz")
            rzt_sb = singles.tile([KROWS, E], f32, tag="rzt")
            nc.sync.dma_start(out=rz_sb, in_=rz[:, :])
            nc.sync.dma_start(out=rzt_sb, in_=rzt[:, :])

            # --- KL: row sums of squares of this core's odd_weights slice ---
            klp_sb = singles.tile([128, 8], f32, tag="klp")
            for rep in range(repeat):
              for i in range(KL_ROWS // 128):
                kt = klt.tile([128, E], f32, tag="klw")
                nc.sync.dma_start(out=kt, in_=klw[128 * i : 128 * (i + 1), :])
                for j in range(4):
                    scr = klt.tile([128, 512], f32, tag="klscr")
                    nc.scalar.activation(
                        out=scr,
                        in_=kt[:, 512 * j : 512 * (j + 1)],
                        func=mybir.ActivationFunctionType.Square,
                        accum_out=klp_sb[:, 4 * i + j : 4 * i + j + 1],
                    )
            nc.sync.dma_start(out=klp[:, :], in_=klp_sb)

            # --- main pipeline over batch quarters ---
            for rep, q in [(r, qq) for r in range(repeat) for qq in range(4)]:
                xq = xqp.tile([KROWS, N_WIN, QCOLS], f32, tag="xq")
                src = bass.AP(
                    tensor=xl,
                    offset=QCOLS * q,
                    ap=[[B_CORE, KROWS], [KROWS * B_CORE, N_WIN], [1, QCOLS]],
                )
                nc.sync.dma_start(out=xq, in_=src)

                for bb in range(QCOLS // 128):
                    row0 = q * QCOLS + bb * 128
                    to_z = outs.tile([128, E], f32, tag="oz")
                    to_zt = outs.tile([128, E], f32, tag="ozt")
                    for w4 in range(4):
                        pz = psum.tile([128, 512], f32, tag="pz")
                        pzt = psum.tile([128, 512], f32, tag="pzt")
                        for w8 in range(8):
                            w = w4 * 8 + w8
                            lhs = xq[:, w, 128 * bb : 128 * (bb + 1)]
                            nc.tensor.matmul(
                                pz[:, WIN * w8 : WIN * (w8 + 1)],
                                lhsT=lhs,
                                rhs=rz_sb[:, WIN * w : WIN * (w + 1)],
                                start=True,
                                stop=True,
                            )
                            nc.tensor.matmul(
                                pzt[:, WIN * w8 : WIN * (w8 + 1)],
                                lhsT=lhs,
                                rhs=rzt_sb[:, WIN * w : WIN * (w + 1)],
                                start=True,
                                stop=True,
                            )
                        nc.scalar.activation(
                            out=to_z[:, 512 * w4 : 512 * (w4 + 1)],
                            in_=pz,
                            func=mybir.ActivationFunctionType.Tanh,
                            scale=0.5,
                        )
                        nc.scalar.activation(
                            out=to_zt[:, 512 * w4 : 512 * (w4 + 1)],
                            in_=pzt,
                            func=mybir.ActivationFunctionType.Tanh,
                            scale=0.5,
                        )
                    # clip in tanh space: min(., tanh5) then max(., -tanh5)
                    nc.vector.tensor_scalar(
                        to_z, to_z, TANH5, -TANH5,
                        mybir.AluOpType.min, mybir.AluOpType.max,
                    )
                    nc.vector.tensor_scalar(
                        to_zt, to_zt, TANH5, -TANH5,
                        mybir.AluOpType.min, mybir.AluOpType.max,
                    )
                    nc.sync.dma_start(out=oz[row0 : row0 + 128, :], in_=to_z)
                    nc.sync.dma_start(out=ozt[row0 : row0 + 128, :], in_=to_zt)

    nc.compile()
    return nc


def _get_bass(repeat=1):
    global _CACHED
    if _CACHED is None:
        _CACHED = {}
    if repeat not in _CACHED:
        _CACHED[repeat] = _build_bass(repeat=repeat)
    return _CACHED[repeat]


def _sigmoid64(v):
    return 1.0 / (1.0 + np.exp(-v.astype(np.float64)))


def _prepare_in_maps(x, llr, odd_weights, llr_weights, dropout_logit, u,
                     w_odd2even_mask, w_skipconn2even_mask):
    x = np.ascontiguousarray(np.asarray(x, dtype=np.float32))
    llr = np.ascontiguousarray(np.asarray(llr, dtype=np.float32))
    odd_weights = np.ascontiguousarray(np.asarray(odd_weights, dtype=np.float32))
    llr_weights = np.asarray(llr_weights, dtype=np.float32)
    dropout_logit = np.asarray(dropout_logit, dtype=np.float32)
    u = np.asarray(u, dtype=np.float32)
    w_odd2even_mask = np.asarray(w_odd2even_mask, dtype=np.float32)
    w_skipconn2even_mask = np.asarray(w_skipconn2even_mask, dtype=np.float32)

    # Dropout gates.  p in float64->float32 matches jax's fp32 sigmoid to
    # <=1 ulp; the nearest |u - p| gap in this problem is ~2e-5, so the
    # comparisons cannot flip.
    p = _sigmoid64(dropout_logit).astype(np.float32)
    pneg = _sigmoid64(-dropout_logit).astype(np.float32)

    # Per-window masked weights (only the block diagonal is ever nonzero).
    rhs_z = np.zeros((KROWS, E), dtype=np.float32)
    rhs_zt = np.zeros((KROWS, E), dtype=np.float32)
    for w in range(N_WIN):
        sl = slice(WIN * w, WIN * (w + 1))
        vs = slice((WIN // DV) * w, (WIN // DV) * (w + 1))
        tm = w_odd2even_mask[sl, sl] * odd_weights[sl, sl]
        z = (u[sl, sl] < p[None, sl]).astype(np.float32)
        zt = (u[sl, sl] > pneg[None, sl]).astype(np.float32)
        rhs_z[:WIN, sl] = tm * z
        rhs_zt[:WIN, sl] = tm * zt
        wl = w_skipconn2even_mask[vs, sl] * llr_weights[vs, sl]
        rhs_z[WIN:, sl] = wl
        rhs_zt[WIN:, sl] = wl

    # Pack transposed x / llr shards: xl[c] is [N_WIN, 80, B_CORE].
    xr = np.transpose(x.reshape(N_CORES, B_CORE, N_WIN, WIN), (0, 2, 3, 1))
    lr = np.transpose(llr.reshape(N_CORES, B_CORE, N_WIN, WIN // DV), (0, 2, 3, 1))
    xl = np.concatenate([xr, lr], axis=2).reshape(N_CORES, N_WIN * KROWS, B_CORE)
    xl = np.ascontiguousarray(xl)

    in_maps = []
    for c in range(N_CORES):
        in_maps.append({
            "xl": xl[c],
            "rhs_z": rhs_z,
            "rhs_zt": rhs_zt,
            "klw": np.ascontiguousarray(odd_weights[KL_ROWS * c : KL_ROWS * (c + 1)]),
        })
    return in_maps


def kernel(x, llr, odd_weights, llr_weights, dropout_logit, u,
           w_odd2even_mask, w_skipconn2even_mask):
    dropout_logit = np.asarray(dropout_logit, dtype=np.float32)
    in_maps = _prepare_in_maps(x, llr, odd_weights, llr_weights, dropout_logit,
                               u, w_odd2even_mask, w_skipconn2even_mask)
    global LAST_IN_MAPS, LAST_RESULT
    LAST_IN_MAPS = in_maps

    from concourse.bass_utils import run_bass_kernel_spmd

    nc = _get_bass()
    trace = bool(int(os.environ.get("KERNEL_TRACE", "0")))
    res = run_bass_kernel_spmd(nc, in_maps, core_ids=list(range(N_CORES)),
                               trace=trace)
    LAST_RESULT = res

    output = np.empty((B, E), dtype=np.float32)
    out_tilde = np.empty((B, E), dtype=np.float32)
    rowsumsq = np.empty(E, dtype=np.float64)
    for c in range(N_CORES):
        r = res.results[c]
        output[B_CORE * c : B_CORE * (c + 1)] = r["oz"]
        out_tilde[B_CORE * c : B_CORE * (c + 1)] = r["ozt"]
        kp = r["klp"].astype(np.float64)
        rowsumsq[KL_ROWS * c : KL_ROWS * c + 128] = kp[:, 0:4].sum(axis=1)
        rowsumsq[KL_ROWS * c + 128 : KL_ROWS * (c + 1)] = kp[:, 4:8].sum(axis=1)

    p64 = _sigmoid64(dropout_logit)
    scaling1 = (KL_SCALE ** 2 / 2.0) * p64
    h1 = -p64 * np.log(p64) - (1.0 - p64) * np.log(1.0 - p64)
    kl_term = np.float32(np.mean(scaling1 * rowsumsq - h1))

    return output, out_tilde, kl_term


def time_device(inputs=None, iters=30, repeat=1):
    """Median wall-clock (s) of one sharded-NEFF execution RPC.

    Device-resident inputs, reused (non-donated) output buffers; our kernel
    writes every output element so output-buffer reuse is safe.  The wall
    time is dominated by the axon RPC; use two different `repeat` unroll
    factors and difference them to recover on-device time (see
    measure_hw_ns).
    """
    import time as _time

    import jax
    from jax.sharding import Mesh, NamedSharding, PartitionSpec
    from jax.experimental.shard_map import shard_map

    import concourse.mybir as mybir
    from concourse import bass2jax

    nc = _get_bass(repeat=repeat)
    in_maps = LAST_IN_MAPS
    if in_maps is None:
        if inputs is None:
            raise ValueError("run kernel() first or pass inputs")
        in_maps = _prepare_in_maps(**inputs)

    bass2jax.install_neuronx_cc_hook()
    partition_name = nc.partition_id_tensor.name if nc.partition_id_tensor else None
    in_names, out_names, out_avals, zero_outs = [], [], [], []
    for alloc in nc.m.functions[0].allocations:
        if not isinstance(alloc, mybir.MemoryLocationSet):
            continue
        name = alloc.memorylocations[0].name
        if alloc.kind == "ExternalInput":
            if name != partition_name:
                in_names.append(name)
        elif alloc.kind == "ExternalOutput":
            shape = tuple(alloc.tensor_shape)
            dtype = mybir.dt.np(alloc.dtype)
            out_names.append(name)
            out_avals.append(jax.core.ShapedArray(shape, dtype))
            zero_outs.append(np.zeros(shape, dtype))
    n_params = len(in_names)
    all_names = list(in_names) + list(out_names)
    if partition_name is not None:
        all_names.append(partition_name)

    def _body(*args):
        operands = list(args)
        if partition_name is not None:
            operands.append(bass2jax.partition_id_tensor())
        outs = bass2jax._bass_exec_p.bind(
            *operands,
            out_avals=tuple(out_avals),
            in_names=tuple(all_names),
            out_names=tuple(out_names),
            lowering_input_output_aliases=(),
            sim_require_finite=True,
            sim_require_nnan=True,
            nc=nc,
        )
        return tuple(outs)

    devices = jax.devices()[:N_CORES]
    mesh = Mesh(np.asarray(devices), ("core",))
    spec = PartitionSpec("core")
    nin = n_params + len(out_names)
    fn = jax.jit(
        shard_map(_body, mesh=mesh, in_specs=(spec,) * nin,
                  out_specs=(spec,) * len(out_names), check_rep=False),
        keep_unused=True,
    )
    sh = NamedSharding(mesh, spec)
    args = [
        jax.device_put(
            np.concatenate([np.asarray(in_maps[c][nm]) for c in range(N_CORES)], 0),
            sh,
        )
        for nm in in_names
    ] + [
        jax.device_put(np.zeros((N_CORES * z.shape[0], *z.shape[1:]), z.dtype), sh)
        for z in zero_outs
    ]

    jax.block_until_ready(fn(*args))  # compile + warm
    jax.block_until_ready(fn(*args))
    times = []
    for _ in range(iters):
        t0 = _time.perf_counter()
        jax.block_until_ready(fn(*args))
        times.append(_time.perf_counter() - t0)
    return float(np.median(times))


def measure_hw_ns(inputs=None, iters=12, repeat=9):
    """On-device per-execution time (ns): RPC-cancelled difference between a
    `repeat`-times-unrolled NEFF and the single-shot NEFF."""
    t1 = time_device(inputs, iters=iters, repeat=1)
    tr = time_device(inputs, iters=iters, repeat=repeat)
    return (tr - t1) / (repeat - 1) * 1e9, t1, tr
